# revision 1
# baseline (speedup 1.0000x reference)
"""Trainium2 Bass kernel for nn_MultiHeadAttention_61546881352366.

The reference module's observable output is NOT attention: the attention
result is dead code in the original torch module.  The output is

    out = fc0(concat_h(v @ Wv_h^T)) = (v @ Wcat^T) @ W0^T + b0

with Wcat = Wv.reshape(H*D, C).  Two chained linear maps fuse into one:

    out = v @ (W0 @ Wcat)^T + b0 = v @ WcT + b0,   WcT = (W0 @ Wcat)^T

so the device work is a single [B*T, C] @ [C, C] matmul plus a bias add.
k and q are unused.

Sharding: data-parallel over batch (B == 8 == n_cores); each core computes
one batch element's [2048, 1024] @ [1024, 1024] product.  Weights are
replicated (2 MiB/core in bf16).

Device kernel (per core):
  - inputs: vT [C, T] bf16 (v[b] transposed + cast on host so the
    contraction dim c lands on SBUF partitions), wT = WcT [C, C] bf16,
    bias broadcast [128, C] fp32
  - 256 matmuls of [128x128] @ [128x512] in bf16 (full PE rate, fp32 PSUM
    accumulate; rel err ~2e-3), accumulated over the 8 contraction tiles
  - k-outer fill phase over the first 4 row tiles so the PE never idles
    while weights stream in; m-major steady phase so PSUM->SBUF copies
    pace with compute; PE-warmup matmuls ramp the clock during DMA fill
  - bias add fused into the PSUM->SBUF copy on the vector engine

`reps` repeats the whole body inside one NEFF (same math, output
overwritten) -- used only by the timing harness to amortize launch
overhead; the graded path uses reps=1.
"""

import numpy as np

import concourse.bacc as bacc
import concourse.mybir as mybir
from concourse.tile import TileContext
from concourse.bass_utils import run_bass_kernel_spmd

B, T, C = 8, 2048, 1024
H, D = 16, 64
P = 128
KT = C // P  # 8 contraction tiles
MT = T // P  # 16 row tiles per core
NF = 512     # matmul moving free dim (= one PSUM bank of fp32)
NJ = C // NF  # 2 output column tiles

_FP32R = mybir.dt.float32r
_FP32 = mybir.dt.float32
_BF16 = mybir.dt.bfloat16


KC = 2           # k-tiles per w DMA chunk
NWC = KT // KC   # 4 chunks per j half
N_WARMUP = 6     # dummy matmuls to ramp the PE clock during the DMA fill


def _build(reps=1):
    nc = bacc.Bacc()
    vT = nc.dram_tensor("vT", [C, T], _BF16, kind="ExternalInput")
    wT = nc.dram_tensor("wT", [C, C], _BF16, kind="ExternalInput")
    bias = nc.dram_tensor("bias", [P, C], _FP32, kind="ExternalInput")
    out = nc.dram_tensor("out", [T, C], _FP32, kind="ExternalOutput")

    vT_r = vT[:, :].rearrange("(k p) t -> p k t", p=P)  # [128, KT, T]
    wT_r = wT[:, :].rearrange("(k p) j -> p k j", p=P)  # [128, KT, C]

    with TileContext(nc) as tc:
        with (
            tc.tile_pool(name="wpool", bufs=1) as wpool,
            tc.tile_pool(name="vpool", bufs=12) as vpool,
            tc.tile_pool(name="bpool", bufs=1) as bpool,
            tc.tile_pool(name="opool", bufs=6) as opool,
            tc.tile_pool(name="pspool", bufs=8, space="PSUM") as pspool,
        ):
            # PE warmup: a few dependency-free matmuls on a memset tile so
            # the PE clock ramps while the first DMAs are in flight.
            scratch = bpool.tile([P, NF], _BF16, name="scratch", tag="scratch")
            nc.vector.memset(scratch, 0.0)
            ps_w = pspool.tile([P, NF], _FP32, name="ps_w", tag="ps")
            for _ in range(N_WARMUP):
                nc.tensor.matmul(
                    ps_w, lhsT=scratch[:, :P], rhs=scratch, start=True, stop=True
                )

            if reps == 1:
                _one_pass(nc, tc, vT_r, wT_r, bias, out, wpool, vpool, bpool, opool, pspool)
            else:
                with tc.For_i(0, reps, 1, hint_engines=(mybir.EngineType.PE,)):
                    _one_pass(nc, tc, vT_r, wT_r, bias, out, wpool, vpool, bpool, opool, pspool)
    nc.compile()
    return nc


def _one_pass(nc, tc, vT_r, wT_r, bias, out, wpool, vpool, bpool, opool, pspool):
    # w chunks: w_sb[j][c] covers k-tiles [c*KC, (c+1)*KC) of column half j.
    # DMA issue order interleaves w chunks (j minor) with v strip pairs so
    # the first matmuls' inputs land first.
    w_sb = [[None] * NWC for _ in range(NJ)]
    v_sb = [None] * MT

    def dma_w(j, c):
        w_jc = wpool.tile([P, KC, NF], _BF16, name=f"w_{j}_{c}", tag=f"w_{j}_{c}")
        nc.scalar.dma_start(
            out=w_jc,
            in_=wT_r[:, c * KC : (c + 1) * KC, j * NF : (j + 1) * NF],
        )
        w_sb[j][c] = w_jc

    def dma_v2(mp):
        # one DMA covers m-strips 2*mp and 2*mp+1, all k
        v_p = vpool.tile([P, KT, 2 * P], _BF16, name=f"v_{mp}", tag="v")
        nc.scalar.dma_start(
            out=v_p, in_=vT_r[:, :, mp * 2 * P : (mp + 1) * 2 * P]
        )
        v_sb[2 * mp] = v_p
        v_sb[2 * mp + 1] = v_p

    dma_w(0, 0)
    dma_v2(0)
    dma_w(1, 0)
    dma_v2(1)
    dma_w(0, 1)
    dma_w(1, 1)
    dma_v2(2)
    dma_w(0, 2)
    dma_w(1, 2)
    b_sb = bpool.tile([P, C], _FP32, name="b_sb", tag="b_sb", bufs=2)
    nc.scalar.dma_start(out=b_sb, in_=bias[:, :])
    dma_v2(3)
    dma_w(0, 3)
    dma_w(1, 3)
    for mp in range(4, MT // 2):
        dma_v2(mp)

    def mm(ps_mj, m, k, j):
        nc.tensor.matmul(
            ps_mj,
            lhsT=v_sb[m][:, k, (m % 2) * P : (m % 2 + 1) * P],
            rhs=w_sb[j][k // KC][:, k % KC, :],
            start=(k == 0),
            stop=(k == KT - 1),
        )

    def drain(m, ob, ps):
        for j in range(NJ):
            sl = slice(j * NF, (j + 1) * NF)
            nc.vector.tensor_add(ob[:, sl], ps[j], b_sb[:, sl])
            if m == MT - 1:
                nc.sync.dma_start(out=out[m * P : (m + 1) * P, sl], in_=ob[:, sl])
        if m < MT - 1:
            nc.sync.dma_start(out=out[m * P : (m + 1) * P, :], in_=ob)

    # Fill phase (m0-3): k-outer so each arriving w chunk immediately feeds
    # 16 matmuls -- the PE never idles while w streams in.
    G = 4
    ms = range(G)
    psg = {
        (m, j): pspool.tile([P, NF], _FP32, name=f"ps_{m}_{j}", tag="ps")
        for m in ms
        for j in range(NJ)
    }
    obg = {m: opool.tile([P, C], _FP32, name=f"ob_{m}", tag="ob") for m in ms}
    for c in range(NWC):
        for m in ms:
            for kk in range(KC):
                for j in range(NJ):
                    mm(psg[m, j], m, c * KC + kk, j)
            if c == NWC - 1:
                drain(m, obg[m], [psg[m, j] for j in range(NJ)])

    # Steady phase (m4-15): m-major, copies pace with compute, so no copy
    # pile-up trails the final matmul.
    for m in range(G, MT):
        ob = opool.tile([P, C], _FP32, name=f"ob_{m}", tag="ob")
        ps = [
            pspool.tile([P, NF], _FP32, name=f"ps_{m}_{j}", tag="ps")
            for j in range(NJ)
        ]
        for k in range(KT):
            for j in range(NJ):
                mm(ps[j], m, k, j)
        drain(m, ob, ps)


_nc_cache = None


def _get_nc():
    global _nc_cache
    if _nc_cache is None:
        _nc_cache = _build()
    return _nc_cache


def prepare_inputs(inputs):
    """Host-side prep shared by kernel() and the timing harness."""
    import ml_dtypes

    v = np.ascontiguousarray(np.asarray(inputs["v"], dtype=np.float32))
    Wv = np.asarray(inputs["Wv"], dtype=np.float32)
    W0 = np.asarray(inputs["W0"], dtype=np.float32)
    b0 = np.asarray(inputs["b0"], dtype=np.float32)

    # Fuse the two linear layers on the host: WcT[c, j] = (W0 @ Wcat)[j, c]^T
    Wc = W0 @ Wv.reshape(H * D, C)  # [C_out, C_in]
    wT = np.ascontiguousarray(Wc.T.astype(ml_dtypes.bfloat16))  # [C_in, C_out]
    bias = np.ascontiguousarray(
        np.broadcast_to(b0[None, :], (P, C)).astype(np.float32)
    )
    vT = np.ascontiguousarray(
        v.transpose(0, 2, 1).astype(ml_dtypes.bfloat16)
    )  # [B, C, T]
    return [{"vT": vT[i], "wT": wT, "bias": bias} for i in range(B)]


def kernel(**inputs):
    in_maps = prepare_inputs(inputs)
    nc = _get_nc()
    res = run_bass_kernel_spmd(nc, in_maps, core_ids=list(range(B)))
    return np.stack([res.results[i]["out"] for i in range(B)], axis=0)



# revision 2
# speedup vs baseline: 1.0150x; 1.0150x over previous
"""Trainium2 Bass kernel for nn_MultiHeadAttention_61546881352366 (v2).

Math (attention in the reference is dead code):

    out = v @ (W0 @ Wv.reshape(C, C))^T + b0  =  v @ WcT + b0

Per core (data-parallel over batch): one [2048, 1024] @ [1024, 1024] bf16
matmul + bias.  v2 changes vs v1:

  - v is host-packed per m-pair strip ([128, KT*256] contiguous per
    partition) so each v DMA uses 4KB descriptors instead of 512B --
    early strips land ~3x sooner, removing the fill-phase PE gaps.
  - w is host-packed per-k ([KT, 128, C], 2KB rows); fill phase is
    k-outer with KC=1 so the first matmuls need only w_k0 + v-pair 0.
  - output is written bf16 (host upconverts to fp32; adds ~0.2% rms
    error, well inside the 2e-2 gate) halving output DMA bytes.
  - the last m-tile drains per j-half so the final ADD+DMA tail after
    the last matmul is ~1.5us instead of ~4.5us.
"""

import numpy as np

import concourse.bacc as bacc
import concourse.mybir as mybir
from concourse.tile import TileContext
from concourse.bass_utils import run_bass_kernel_spmd

B, T, C = 8, 2048, 1024
H, D = 16, 64
P = 128
KT = C // P       # 8 contraction tiles
MT = T // P       # 16 row tiles per core
MP = MT // 2      # 8 v pair strips
TV = 2 * P        # 256 tokens per v strip
NF = 512          # matmul moving free dim (= one PSUM bank of fp32)
NJ = C // NF      # 2 output column tiles

_FP32 = mybir.dt.float32
_BF16 = mybir.dt.bfloat16

N_WARMUP = 6      # dummy matmuls to ramp the PE clock during the DMA fill
G = 4             # fill-phase row tiles (k-outer, bounded by 8 PSUM banks)


def _build():
    nc = bacc.Bacc()
    vP = nc.dram_tensor("vP", [MP, P, KT * TV], _BF16, kind="ExternalInput")
    wP = nc.dram_tensor("wP", [KT, P, C], _BF16, kind="ExternalInput")
    bias = nc.dram_tensor("bias", [P, C], _FP32, kind="ExternalInput")
    out = nc.dram_tensor("out", [T, C], _BF16, kind="ExternalOutput")

    with TileContext(nc) as tc:
        with (
            tc.tile_pool(name="wpool", bufs=1) as wpool,
            tc.tile_pool(name="vpool", bufs=8) as vpool,
            tc.tile_pool(name="bpool", bufs=1) as bpool,
            tc.tile_pool(name="opool", bufs=6) as opool,
            tc.tile_pool(name="pspool", bufs=8, space="PSUM") as pspool,
        ):
            # PE warmup: dependency-free matmuls on a memset tile so the
            # PE clock ramps while the first DMAs are in flight.
            scratch = bpool.tile([P, NF], _BF16, name="scratch", tag="scratch")
            nc.vector.memset(scratch, 0.0)
            ps_w = pspool.tile([P, NF], _FP32, name="ps_w", tag="ps")
            for _ in range(N_WARMUP):
                nc.tensor.matmul(
                    ps_w, lhsT=scratch[:, :P], rhs=scratch, start=True, stop=True
                )

            w_sb = [None] * KT
            v_sb = [None] * MP

            def dma_w(k):
                w_k = wpool.tile([P, C], _BF16, name=f"w_{k}", tag=f"w_{k}")
                nc.scalar.dma_start(out=w_k, in_=wP[k])
                w_sb[k] = w_k

            def dma_v(mp):
                v_p = vpool.tile([P, KT, TV], _BF16, name=f"v_{mp}", tag="v")
                nc.scalar.dma_start(out=v_p, in_=vP[mp])
                v_sb[mp] = v_p

            # Issue order: first matmuls (k-outer fill over m0-3) need
            # w_k0 + v0 only, then w_k1, v1, ...  Everything needed for
            # fill step k arrives well before the PE reaches it.
            dma_w(0)
            dma_v(0)
            dma_w(1)
            dma_v(1)
            dma_w(2)
            dma_v(2)
            dma_w(3)
            b_sb = bpool.tile([P, C], _FP32, name="b_sb", tag="b_sb")
            nc.scalar.dma_start(out=b_sb, in_=bias[:, :])
            dma_w(4)
            dma_v(3)
            dma_w(5)
            dma_w(6)
            dma_w(7)
            for mp in range(4, MP):
                dma_v(mp)

            def mm(ps_mj, m, k, j):
                nc.tensor.matmul(
                    ps_mj,
                    lhsT=v_sb[m // 2][:, k, (m % 2) * P : (m % 2 + 1) * P],
                    rhs=w_sb[k][:, j * NF : (j + 1) * NF],
                    start=(k == 0),
                    stop=(k == KT - 1),
                )

            def drain(m, ob, ps):
                for j in range(NJ):
                    sl = slice(j * NF, (j + 1) * NF)
                    nc.vector.tensor_add(ob[:, sl], ps[j], b_sb[:, sl])
                nc.sync.dma_start(out=out[m * P : (m + 1) * P, :], in_=ob)

            # Fill phase (m0-3): k-outer so each arriving (w_k, v) pair
            # immediately feeds matmuls -- the PE never idles while
            # inputs stream in.
            psg = {
                (m, j): pspool.tile([P, NF], _FP32, name=f"ps_{m}_{j}", tag="ps")
                for m in range(G)
                for j in range(NJ)
            }
            obg = {
                m: opool.tile([P, C], _BF16, name=f"ob_{m}", tag="ob")
                for m in range(G)
            }
            for k in range(KT):
                for m in range(G):
                    for j in range(NJ):
                        mm(psg[m, j], m, k, j)
                    if k == KT - 1:
                        drain(m, obg[m], [psg[m, j] for j in range(NJ)])

            # Steady phase (m4-14): m-major, copies pace with compute.
            for m in range(G, MT - 1):
                ob = opool.tile([P, C], _BF16, name=f"ob_{m}", tag="ob")
                ps = [
                    pspool.tile([P, NF], _FP32, name=f"ps_{m}_{j}", tag="ps")
                    for j in range(NJ)
                ]
                for k in range(KT):
                    for j in range(NJ):
                        mm(ps[j], m, k, j)
                drain(m, ob, ps)

            # Last m-tile: j-split so the j0 drain overlaps the j1
            # matmuls and only one ADD + half-row DMA trails the final
            # matmul.
            m = MT - 1
            ob = opool.tile([P, C], _BF16, name=f"ob_{m}", tag="ob")
            for j in range(NJ):
                ps_j = pspool.tile([P, NF], _FP32, name=f"ps_{m}_{j}", tag="ps")
                for k in range(KT):
                    mm(ps_j, m, k, j)
                sl = slice(j * NF, (j + 1) * NF)
                nc.vector.tensor_add(ob[:, sl], ps_j, b_sb[:, sl])
                nc.sync.dma_start(out=out[m * P : (m + 1) * P, sl], in_=ob[:, sl])
    nc.compile()
    return nc


_nc_cache = None


def _get_nc():
    global _nc_cache
    if _nc_cache is None:
        _nc_cache = _build()
    return _nc_cache


def prepare_inputs(inputs):
    """Host-side prep shared by kernel() and the timing harness."""
    import ml_dtypes

    v = np.ascontiguousarray(np.asarray(inputs["v"], dtype=np.float32))
    Wv = np.asarray(inputs["Wv"], dtype=np.float32)
    W0 = np.asarray(inputs["W0"], dtype=np.float32)
    b0 = np.asarray(inputs["b0"], dtype=np.float32)

    # Fuse the two linear layers on the host: Wc = W0 @ Wcat, [C_out, C_in]
    Wc = W0 @ Wv.reshape(H * D, C)
    # wP[k, p, j] = Wc[j, k*128+p]
    wP = np.ascontiguousarray(
        Wc.T.reshape(KT, P, C).astype(ml_dtypes.bfloat16)
    )
    bias = np.ascontiguousarray(
        np.broadcast_to(b0[None, :], (P, C)).astype(np.float32)
    )
    # vP[b, mp, p, k*256+tt] = v[b, mp*256+tt, k*128+p]
    vb = v.astype(ml_dtypes.bfloat16)
    vP = np.ascontiguousarray(
        vb.reshape(B, MP, TV, KT, P).transpose(0, 1, 4, 3, 2).reshape(
            B, MP, P, KT * TV
        )
    )
    return [{"vP": vP[i], "wP": wP, "bias": bias} for i in range(B)]


def kernel(**inputs):
    in_maps = prepare_inputs(inputs)
    nc = _get_nc()
    res = run_bass_kernel_spmd(nc, in_maps, core_ids=list(range(B)))
    return np.stack(
        [res.results[i]["out"].astype(np.float32) for i in range(B)], axis=0
    )


# revision 3
# speedup vs baseline: 1.0789x; 1.0630x over previous
"""Trainium2 Bass kernel for nn_MultiHeadAttention_61546881352366 (v2).

Math (attention in the reference is dead code):

    out = v @ (W0 @ Wv.reshape(C, C))^T + b0  =  v @ WcT + b0

Per core (data-parallel over batch): one [2048, 1024] @ [1024, 1024] bf16
matmul + bias.  v2 changes vs v1:

  - v is host-packed per m-pair strip ([128, KT*256] contiguous per
    partition) so each v DMA uses 4KB descriptors instead of 512B --
    early strips land ~3x sooner, removing the fill-phase PE gaps.
  - w is host-packed per-k ([KT, 128, C], 2KB rows); fill phase is
    k-outer with KC=1 so the first matmuls need only w_k0 + v-pair 0.
  - output is written bf16 (host upconverts to fp32; adds ~0.2% rms
    error, well inside the 2e-2 gate) halving output DMA bytes.
  - the last m-tile drains per j-half so the final ADD+DMA tail after
    the last matmul is ~1.5us instead of ~4.5us.
"""

import numpy as np

import concourse.bacc as bacc
import concourse.mybir as mybir
from concourse.tile import TileContext
from concourse.bass_utils import run_bass_kernel_spmd

B, T, C = 8, 2048, 1024
H, D = 16, 64
P = 128
KT = C // P       # 8 contraction tiles
MT = T // P       # 16 row tiles per core
MP = MT // 2      # 8 v pair strips
TV = 2 * P        # 256 tokens per v strip
NF = 512          # matmul moving free dim (= one PSUM bank of fp32)
NJ = C // NF      # 2 output column tiles

_FP32 = mybir.dt.float32
_BF16 = mybir.dt.bfloat16

N_WARMUP = 6      # dummy matmuls to ramp the PE clock during the DMA fill
G = 4             # fill-phase row tiles (k-outer, bounded by 8 PSUM banks)


def _build():
    nc = bacc.Bacc()
    vP = nc.dram_tensor("vP", [MP, P, KT * TV], _BF16, kind="ExternalInput")
    wP = nc.dram_tensor("wP", [KT, P, C], _BF16, kind="ExternalInput")
    bias = nc.dram_tensor("bias", [P, C], _FP32, kind="ExternalInput")
    out = nc.dram_tensor("out", [T, C], _BF16, kind="ExternalOutput")

    with TileContext(nc) as tc:
        with (
            tc.tile_pool(name="wpool", bufs=1) as wpool,
            tc.tile_pool(name="vpool", bufs=8) as vpool,
            tc.tile_pool(name="bpool", bufs=1) as bpool,
            tc.tile_pool(name="opool", bufs=6) as opool,
            tc.tile_pool(name="pspool", bufs=8, space="PSUM") as pspool,
        ):
            # PE warmup: dependency-free matmuls on a memset tile so the
            # PE clock ramps while the first DMAs are in flight.
            scratch = bpool.tile([P, NF], _BF16, name="scratch", tag="scratch")
            nc.vector.memset(scratch, 0.0)
            ps_w = pspool.tile([P, NF], _FP32, name="ps_w", tag="ps")
            for _ in range(N_WARMUP):
                nc.tensor.matmul(
                    ps_w, lhsT=scratch[:, :P], rhs=scratch, start=True, stop=True
                )

            w_sb = [None] * KT
            v_sb = [None] * MP

            def dma_w(k):
                w_k = wpool.tile([P, C], _BF16, name=f"w_{k}", tag=f"w_{k}")
                nc.scalar.dma_start(out=w_k, in_=wP[k])
                w_sb[k] = w_k

            def dma_v(mp):
                v_p = vpool.tile([P, KT, TV], _BF16, name=f"v_{mp}", tag="v")
                nc.scalar.dma_start(out=v_p, in_=vP[mp])
                v_sb[mp] = v_p

            # Issue order: w strips maximize fill-phase work-per-byte
            # (each 256KB w_k unlocks 8 matmuls once v0/v1 are in), so
            # after the first two v pairs stream ALL w, then the rest
            # of v for the steady phase.
            dma_w(0)
            dma_v(0)
            dma_w(1)
            dma_v(1)
            for k in range(2, KT):
                dma_w(k)
            b_sb = bpool.tile([P, C], _FP32, name="b_sb", tag="b_sb")
            nc.scalar.dma_start(out=b_sb, in_=bias[:, :])
            for mp in range(2, MP):
                dma_v(mp)

            def mm(ps_mj, m, k, j):
                nc.tensor.matmul(
                    ps_mj,
                    lhsT=v_sb[m // 2][:, k, (m % 2) * P : (m % 2 + 1) * P],
                    rhs=w_sb[k][:, j * NF : (j + 1) * NF],
                    start=(k == 0),
                    stop=(k == KT - 1),
                )

            def drain(m, ob, ps):
                for j in range(NJ):
                    sl = slice(j * NF, (j + 1) * NF)
                    nc.vector.tensor_add(ob[:, sl], ps[j], b_sb[:, sl])
                nc.sync.dma_start(out=out[m * P : (m + 1) * P, :], in_=ob)

            # Fill phase (m0-3): ordered to match DMA arrival so the
            # in-order tensor sequencer never stalls on a tile that is
            # behind others in the stream: k0/k1 for m0-1 (needs only
            # w0,v0,w1), then k0/k1 for m2-3 (v1), then k2..k7 across
            # all four m.
            psg = {
                (m, j): pspool.tile([P, NF], _FP32, name=f"ps_{m}_{j}", tag="ps")
                for m in range(G)
                for j in range(NJ)
            }
            obg = {
                m: opool.tile([P, C], _BF16, name=f"ob_{m}", tag="ob")
                for m in range(G)
            }

            def fill(ms, ks):
                for k in ks:
                    for m in ms:
                        for j in range(NJ):
                            mm(psg[m, j], m, k, j)
                        if k == KT - 1:
                            drain(m, obg[m], [psg[m, j] for j in range(NJ)])

            fill((0, 1), (0,))
            fill((0, 1), (1,))
            fill((2, 3), (0, 1))
            fill((0, 1, 2, 3), range(2, KT))

            # Steady phase (m4-14): m-major, copies pace with compute.
            for m in range(G, MT - 1):
                ob = opool.tile([P, C], _BF16, name=f"ob_{m}", tag="ob")
                ps = [
                    pspool.tile([P, NF], _FP32, name=f"ps_{m}_{j}", tag="ps")
                    for j in range(NJ)
                ]
                for k in range(KT):
                    for j in range(NJ):
                        mm(ps[j], m, k, j)
                drain(m, ob, ps)

            # Last m-tile: j-split so the j0 drain overlaps the j1
            # matmuls and only one ADD + half-row DMA trails the final
            # matmul.
            m = MT - 1
            ob = opool.tile([P, C], _BF16, name=f"ob_{m}", tag="ob")
            for j in range(NJ):
                ps_j = pspool.tile([P, NF], _FP32, name=f"ps_{m}_{j}", tag="ps")
                for k in range(KT):
                    mm(ps_j, m, k, j)
                sl = slice(j * NF, (j + 1) * NF)
                nc.vector.tensor_add(ob[:, sl], ps_j, b_sb[:, sl])
                nc.sync.dma_start(out=out[m * P : (m + 1) * P, sl], in_=ob[:, sl])
    nc.compile()
    return nc


_nc_cache = None


def _get_nc():
    global _nc_cache
    if _nc_cache is None:
        _nc_cache = _build()
    return _nc_cache


def prepare_inputs(inputs):
    """Host-side prep shared by kernel() and the timing harness."""
    import ml_dtypes

    v = np.ascontiguousarray(np.asarray(inputs["v"], dtype=np.float32))
    Wv = np.asarray(inputs["Wv"], dtype=np.float32)
    W0 = np.asarray(inputs["W0"], dtype=np.float32)
    b0 = np.asarray(inputs["b0"], dtype=np.float32)

    # Fuse the two linear layers on the host: Wc = W0 @ Wcat, [C_out, C_in]
    Wc = W0 @ Wv.reshape(H * D, C)
    # wP[k, p, j] = Wc[j, k*128+p]
    wP = np.ascontiguousarray(
        Wc.T.reshape(KT, P, C).astype(ml_dtypes.bfloat16)
    )
    bias = np.ascontiguousarray(
        np.broadcast_to(b0[None, :], (P, C)).astype(np.float32)
    )
    # vP[b, mp, p, k*256+tt] = v[b, mp*256+tt, k*128+p]
    vb = v.astype(ml_dtypes.bfloat16)
    vP = np.ascontiguousarray(
        vb.reshape(B, MP, TV, KT, P).transpose(0, 1, 4, 3, 2).reshape(
            B, MP, P, KT * TV
        )
    )
    return [{"vP": vP[i], "wP": wP, "bias": bias} for i in range(B)]


def kernel(**inputs):
    in_maps = prepare_inputs(inputs)
    nc = _get_nc()
    res = run_bass_kernel_spmd(nc, in_maps, core_ids=list(range(B)))
    return np.stack(
        [res.results[i]["out"].astype(np.float32) for i in range(B)], axis=0
    )
